# revision 17
# baseline (speedup 1.0000x reference)
"""Causal multi-head attention (B=4, T=2048, E=1024, H=16, D=64) on 8 TRN2 cores.

Sharding: core c = (batch b = c//2, head-half h = c%2). Each core computes the
QKV projections and attention for its 8 heads over the full sequence, then the
output projection for its 512-column slice of E over the full sequence (the
wot/bo inputs are column-sliced per core on the host, so the program is
uniform across cores). The attention outputs (att = softmax(QK^T)V, laid out
[head_dim, t]) are exchanged between the two cores of a batch with one
AllGather per head pair (bf16, 512 KB), issued as soon as that pair finishes
so the wire time hides behind the remaining attention work.

Scores are computed transposed (S^T[tk, tq]) so the softmax needs no
transposes: exp without max-subtraction (scores are O(10) bounded),
denominators from an augmented ones-block in the V operand, normalization by
reciprocal-multiply. Projections run with bf16 x and f32r weights; QK^T runs
in f32r; AV and the output projection run in bf16.
"""

import numpy as np

_B, _T, _E, _H, _D = 4, 2048, 1024, 16, 64
_NCORES = 8
_TB = 128          # t block
_NBLK = _T // _TB  # 16 global blocks
_MYE = _E // 2     # output columns per core


def _build_nc(repeats=1, local_cc=False):
    import concourse.mybir as mybir
    import concourse.tile as tile
    from concourse import bacc

    f32 = mybir.dt.float32
    f32r = mybir.dt.float32r
    bf16 = mybir.dt.bfloat16
    EXP = mybir.ActivationFunctionType.Exp

    nc = bacc.Bacc("TRN2", target_bir_lowering=False, debug=False,
                   num_devices=_NCORES)

    xt_d = nc.dram_tensor("xt", [_E, _T], f32, kind="ExternalInput").ap()
    wk_d = nc.dram_tensor("wk", [_E, 512], f32, kind="ExternalInput").ap()
    wv_d = nc.dram_tensor("wv", [_E, 512], f32, kind="ExternalInput").ap()
    wq_d = nc.dram_tensor("wq", [_E, 512], f32, kind="ExternalInput").ap()
    wot_d = nc.dram_tensor("wot", [_E, _MYE], f32, kind="ExternalInput").ap()
    bo_d = nc.dram_tensor("bo_b", [128, _MYE], f32, kind="ExternalInput").ap()
    tri_d = nc.dram_tensor("tri", [128, 128], f32, kind="ExternalInput").ap()
    y_d = nc.dram_tensor("y", [_T, _MYE], f32, kind="ExternalOutput").ap()

    xt_r = xt_d.rearrange("(e p) t -> p e t", p=128)      # [128, 8, 2048]
    wk_r = wk_d.rearrange("(e p) m -> p e m", p=128)      # [128, 8, 512]
    wv_r = wv_d.rearrange("(e p) m -> p e m", p=128)
    wq_r = wq_d.rearrange("(e p) m -> p e m", p=128)

    with tile.TileContext(nc) as tc:
        with (
            tc.tile_pool(name="big", bufs=1) as big,
            tc.tile_pool(name="strm", bufs=2) as strm,
            tc.tile_pool(name="ptp", bufs=2) as ptp,
            tc.tile_pool(name="sml", bufs=2) as sml,
            tc.tile_pool(name="nrm", bufs=2) as nrm,
            tc.tile_pool(name="dram", bufs=1, space="DRAM") as dram,
            tc.tile_pool(name="ps", bufs=2, space="PSUM") as ps,
        ):
          for _rep in range(repeats):
              # x^T resident for the whole kernel: [128, 8 echunks, 2048] bf16,
              # DMA'd in 512-column blocks so the first projection tile only
              # waits for block 0
              xt = big.tile([128, 8, _T], bf16, tag="xt")

              def xt_dma(cb):
                  nc.gpsimd.dma_start(xt[:, :, 512 * cb:512 * cb + 512],
                                      xt_r[:, :, 512 * cb:512 * cb + 512])

              def load_quad_weights(qd, first=False):
                  # DMA-queue order matters at startup: K weights, then the
                  # first x^T block, then the rest interleaved by first use.
                  wk_t = [strm.tile([128, 8, 128], bf16, tag="wk", name="wk_t") for _ in range(2)]
                  wq_t = [strm.tile([128, 8, 128], bf16, tag="wq", name="wq_t") for _ in range(2)]
                  for s2 in range(2):
                      col = 256 * qd + 128 * s2
                      nc.gpsimd.dma_start(wk_t[s2][:], wk_r[:, :, col:col + 128])
                  if first:
                      xt_dma(0)
                  for s2 in range(2):
                      col = 256 * qd + 128 * s2
                      nc.gpsimd.dma_start(wq_t[s2][:], wq_r[:, :, col:col + 128])
                  if first:
                      xt_dma(1)
                  wv_t = big.tile([128, 8, 256], bf16, tag="wv", bufs=2)
                  nc.gpsimd.dma_start(wv_t[:], wv_r[:, :, 256 * qd:256 * qd + 256])
                  if first:
                      xt_dma(2)
                      xt_dma(3)
                  return wk_t, wq_t, wv_t

              wtiles = load_quad_weights(0, first=True)

              # constants + output-proj weights (prefetched at start; small)
              tri_t = big.tile([128, 128], bf16, tag="tri")
              nc.gpsimd.dma_start(tri_t[:], tri_d)
              wot_t = big.tile([128, 8, _MYE], bf16, tag="wot", name="wot_t")
              nc.gpsimd.dma_start(
                  wot_t[:], wot_d.rearrange("(p pp) e -> pp p e", pp=128))
              bo_t = big.tile([128, _MYE], f32, tag="bo", name="bo_t")
              nc.sync.dma_start(bo_t[:], bo_d[:, :])

              # gathered attention outputs (all 16 heads): SBUF tiles fed from
              # the AllGather DRAM buffers; global pair gp = 4*rank_in_pair + p
              attg = [big.tile([128, _T], bf16, tag=f"attg{gp}", name=f"attg{gp}")
                      for gp in range(8)]
              # one exchange per (pair, t-half): the first half of a pair's
              # att is complete at mid-pair, so its wire time hides deeper
              cc_in = [dram.tile([128, _T // 2], bf16, tag=f"cci{i}", name=f"cc_in{i}")
                       for i in range(8)]
              cc_out = [dram.tile([2, 128, _T // 2], bf16, tag=f"cco{i}", name=f"cc_out{i}")
                        for i in range(8)]

              def exchange_half(att_t, p_idx, hf):
                  i = 2 * p_idx + hf
                  cols = slice(1024 * hf, 1024 * hf + 1024)
                  nc.sync.dma_start(cc_in[i][:], att_t[:, cols])
                  if local_cc:
                      # collective-free stand-in (TimelineSim / debugging):
                      # copy own att into both slots
                      for s in range(2):
                          nc.gpsimd.dma_start(cc_out[i][s], cc_in[i][:])
                  else:
                      nc.gpsimd.collective_compute(
                          "AllGather",
                          mybir.AluOpType.bypass,
                          replica_groups=[[0, 1], [2, 3], [4, 5], [6, 7]],
                          ins=[cc_in[i].opt()],
                          outs=[cc_out[i].opt()],
                      )
                  for s in range(2):
                      nc.sync.dma_start(attg[4 * s + p_idx][:, cols],
                                        cc_out[i][s])

              for qd in range(2):  # head quads: my heads 4qd .. 4qd+3
                  wk_t, wq_t, wv_t = wtiles

                  # ---- projections (contract E in 8 chunks of 128) ----
                  ktq = [big.tile([128, _T], f32r, tag=f"kt{s2}", name="ktq", bufs=2) for s2 in range(2)]
                  qtq = [big.tile([128, _T], f32r, tag=f"qt{s2}", name="qtq", bufs=2) for s2 in range(2)]
                  vaq = [big.tile([128, 512], bf16, tag=f"va{tb}", name="vaq") for tb in range(16)]

                  # kT / qT: [128 hd, 2048 t] per pair slice
                  for s2 in range(2):
                      for tcc in range(4):
                          pk = ps.tile([128, 512], f32, tag="proj", bufs=2)
                          for e in range(8):
                              nc.tensor.matmul(
                                  pk[:], wk_t[s2][:, e],
                                  xt[:, e, 512 * tcc:512 * tcc + 512],
                                  start=(e == 0), stop=(e == 7))
                          nc.vector.tensor_copy(
                              ktq[s2][:, 512 * tcc:512 * tcc + 512], pk[:])
                  for s2 in range(2):
                      for tcc in range(4):
                          pq = ps.tile([128, 512], f32, tag="proj", bufs=2)
                          for e in range(8):
                              nc.tensor.matmul(
                                  pq[:], wq_t[s2][:, e],
                                  xt[:, e, 512 * tcc:512 * tcc + 512],
                                  start=(e == 0), stop=(e == 7))
                          nc.vector.tensor_copy(
                              qtq[s2][:, 512 * tcc:512 * tcc + 512], pq[:])

                  # v: per t-block [128 t, 4 heads x (64 v | 64 ones)]
                  for tb in range(16):
                      pv = ps.tile([128, 512], f32, tag="proj", bufs=2)
                      for e in range(8):
                          nc.tensor.matmul(
                              pv[:, 0:256], xt[:, e, 128 * tb:128 * tb + 128],
                              wv_t[:, e], start=(e == 0), stop=(e == 7))
                      src = pv[:, 0:256].rearrange(
                          "p (h x) -> p h x", h=4)
                      dst = vaq[tb][:].rearrange("p (h z) -> p h z", h=4)
                      nc.vector.tensor_copy(dst[:, :, 0:64], src[:])
                      if qd == 0:
                          # ones columns are identical across quads: write once
                          nc.gpsimd.memset(dst[:, :, 64:128], 1.0)

                  # prefetch next quad's weights; they land during attention
                  if qd < 1:
                      wtiles = load_quad_weights(qd + 1)

                  # ---- attention ----
                  for s2 in range(2):         # pair in quad
                      p_idx = 2 * qd + s2
                      att_t = strm.tile([128, _T], bf16, tag="att", name="att_t")
                      for g in range(8):      # query 256-groups; blocks 2g,2g+1
                          # both heads of the pair interleaved: their K=64 QK
                          # matmuls live in disjoint PE row groups (0-63/64-127)
                          po2 = [ps.tile([128, 512], f32, tag="outp",
                                         name="po2", bufs=3) for _ in range(2)]
                          n_cp = g + 1        # chunk pairs; chunks 0..2g+1
                          for cp in range(n_cp):
                              sc2 = [ps.tile([128, 512], f32, tag="score",
                                             name="sc2", bufs=3) for _ in range(2)]
                              for q2 in range(2):
                                  c = 2 * cp + q2
                                  for hh in range(2):
                                      nc.tensor.matmul(
                                          sc2[hh][:, 256 * q2:256 * q2 + 256],
                                          ktq[s2][64 * hh:64 * hh + 64,
                                                  128 * c:128 * c + 128],
                                          qtq[s2][64 * hh:64 * hh + 64,
                                                  256 * g:256 * g + 256],
                                          start=True, stop=True)
                              for hh in range(2):
                                  pt = ptp.tile([128, 512], bf16, tag="pt", bufs=3)
                                  nc.scalar.activation(
                                      pt[:], sc2[hh][:], EXP, scale=0.125)
                                  if cp == n_cp - 1:
                                      # chunk 2g = diag of block A (cols 0:128);
                                      # chunk 2g+1: invalid vs A (cols 256:384),
                                      # diag of block B (cols 384:512)
                                      nc.vector.tensor_mul(
                                          pt[:, 0:128], pt[:, 0:128], tri_t[:])
                                      nc.gpsimd.memset(pt[:, 256:384], 0.0)
                                      nc.vector.tensor_mul(
                                          pt[:, 384:512],
                                          pt[:, 384:512], tri_t[:])
                                  for q2 in range(2):
                                      c = 2 * cp + q2
                                      nc.tensor.matmul(
                                          po2[hh][:, 0:256],
                                          vaq[c][:, 128 * (2 * s2 + hh):
                                                 128 * (2 * s2 + hh) + 128],
                                          pt[:, 256 * q2:256 * q2 + 256],
                                          start=(cp == 0 and q2 == 0),
                                          stop=(cp == n_cp - 1 and q2 == 1))
                          for hh in range(2):
                              po = po2[hh]
                              sums_rows = po[64:128, 0:256]
                              v_rows = po[0:64, 0:256]
                              sums_t = nrm.tile([64, 256], f32, tag="sums")
                              nc.vector.tensor_copy(sums_t[:], sums_rows)
                              rec_t = nrm.tile([64, 256], f32, tag="rec")
                              nc.vector.reciprocal_approx_fast(rec_t[:], sums_t[:])
                              nc.vector.tensor_mul(
                                  att_t[64 * hh:64 * hh + 64,
                                        256 * g:256 * g + 256],
                                  v_rows, rec_t[:])
                          if g == 3:
                              exchange_half(att_t, p_idx, 0)
                      exchange_half(att_t, p_idx, 1)

              # ---- output projection: y[:, my cols] = att_all @ wot + bo ----
              # accumulation ordered so the latest-arriving pairs come last
              gp_order = [0, 4, 1, 5, 2, 6, 3, 7]
              for tb in range(16):
                  py = ps.tile([128, 512], f32, tag="proj", bufs=2)
                  for i_gp, gp in enumerate(gp_order):
                      # heads 0..7 live in slot 0 (rank 2b), 8..15 in slot 1
                      nc.tensor.matmul(
                          py[:], attg[gp][:, 128 * tb:128 * tb + 128],
                          wot_t[:, gp], start=(i_gp == 0), stop=(i_gp == 7))
                  ysb = sml.tile([128, 512], f32, tag="ysb", bufs=2)
                  nc.vector.tensor_add(ysb[:], py[:], bo_t[:])
                  nc.sync.dma_start(
                      y_d[128 * tb:128 * tb + 128, :], ysb[:])

    nc.compile()
    return nc


_NC_CACHE = {}


def _get_nc(repeats=1, local_cc=False):
    key = (repeats, local_cc)
    if key not in _NC_CACHE:
        _NC_CACHE[key] = _build_nc(repeats, local_cc)
    return _NC_CACHE[key]


def _make_in_maps(x, wq, wk, wv, wo, bo):
    x = np.asarray(x, dtype=np.float32)
    wq = np.asarray(wq, dtype=np.float32)
    wk = np.asarray(wk, dtype=np.float32)
    wv = np.asarray(wv, dtype=np.float32)
    wo = np.asarray(wo, dtype=np.float32)
    bo = np.asarray(bo, dtype=np.float32)

    # [H, E, D] -> [E, H*D]
    wq2 = np.ascontiguousarray(wq.transpose(1, 0, 2).reshape(_E, _H * _D))
    wk2 = np.ascontiguousarray(wk.transpose(1, 0, 2).reshape(_E, _H * _D))
    wv2 = np.ascontiguousarray(wv.transpose(1, 0, 2).reshape(_E, _H * _D))
    wot = np.ascontiguousarray(wo.T)                       # [hd, e_out]
    tri = np.ascontiguousarray(
        np.triu(np.ones((128, 128), dtype=np.float32)))    # tk <= tq

    in_maps = []
    for c in range(_NCORES):
        b, h = c // 2, c % 2
        xt = np.ascontiguousarray(x[b].T)                  # [E, T]
        hs = slice(512 * h, 512 * h + 512)                 # my heads' hd cols
        es = slice(_MYE * h, _MYE * h + _MYE)              # my output E cols
        in_maps.append({
            "xt": xt,
            "wk": np.ascontiguousarray(wk2[:, hs]),
            "wv": np.ascontiguousarray(wv2[:, hs]),
            "wq": np.ascontiguousarray(wq2[:, hs]),
            "wot": np.ascontiguousarray(wot[:, es]),
            "bo_b": np.ascontiguousarray(
                np.broadcast_to(bo[es], (128, _MYE))),
            "tri": tri,
        })
    return in_maps


def kernel(x, wq, wk, wv, wo, bo, _want_results=False, _repeats=1, **_ignored):
    from concourse.bass_utils import run_bass_kernel_spmd

    nc = _get_nc(_repeats)
    in_maps = _make_in_maps(x, wq, wk, wv, wo, bo)
    res = run_bass_kernel_spmd(nc, in_maps, core_ids=list(range(_NCORES)))

    out = _assemble([res.results[c]["y"] for c in range(_NCORES)])
    if _want_results:
        return out, res
    return out


def _assemble(ys):
    out = np.empty((_B, _T, _E), dtype=np.float32)
    for c in range(_NCORES):
        b, h = c // 2, c % 2
        out[b][:, _MYE * h:_MYE * h + _MYE] = ys[c].reshape(_T, _MYE)
    return out
